# revision 13
# baseline (speedup 1.0000x reference)
"""Bass/Trainium2 kernel for nn_BipartiteSoftMatching (8 cores, batch-parallel).

Since r = t//2 the argsort in the reference is irrelevant: src_idx is a full
permutation and unm_idx is empty.  Per batch element the computation reduces to
  m = metric / ||metric||;  scores = m_even @ m_odd^T
  node_idx[i] = argmax_j scores[i, j]
  dst_out[j]  = (x_odd[j] + sum_{i: node_idx[i]=j} x_even[i]) / (1 + count[j])
  out[2j+1]   = dst_out[j];   out[2i] = dst_out[node_idx[i]]

Hardware mapping (v1 rewrite):
  - scores: fp32 matmuls (exact; min top-2 gap on this data is 2.7e-6),
    two i-tiles packed into the 128-row array via tile_position halves.
  - argmax: per 512-chunk, DVE reduce_max straight from PSUM, then an ACT
    copy emits fp16((s - cmax) * 2^14) (exact 0 only at the chunk argmax;
    the 2^14 scale keeps the min gap in fp16-normal range) and DVE
    find_index8 searches 0.0 on 2-byte data.  A tiny one-hot combine picks
    the winning chunk.  This replaces two full fp32 scans per tile.
  - scatter-add, bucketed: tokens are permuted into 16 buckets by dest
    j-tile (256 slots, OOB-padded).  slot = 256*bucket + cross-tile-count
    + within-tile-rank, where the within-tile rank comes from a strict
    upper-triangular prefix matmul (PE) instead of a transpose trick.
    (row, idx) pairs scatter to a DRAM perm table; x_even rows gather back
    bucket-contiguous with OOB rows skipped via bounds_check.
  - phase C: one f32r one-hot matmul pass per half-bucket (tolerance is
    2e-2; f32r's ~2^-12 rounding is plenty), counts ride along as a
    2-column ones matmul.
  - phase D is folded into phase C per j-tile: the transposed one-hot
    gathers dst rows with a second f32r matmul and an indirect scatter
    writes them straight to the even output rows (no DRAM staging buffer).
"""

import numpy as np

import concourse.bacc as bacc
import concourse.bass as bass
import concourse.mybir as mybir
import concourse.tile as tile
from concourse.bass import IndirectOffsetOnAxis
from concourse.bass_utils import run_bass_kernel_spmd
from concourse.masks import make_identity

F32 = mybir.dt.float32
F32R = mybir.dt.float32r
F16 = mybir.dt.float16
U16 = mybir.dt.uint16
U32 = mybir.dt.uint32
I32 = mybir.dt.int32
OP = mybir.AluOpType
AF = mybir.ActivationFunctionType
AX = mybir.AxisListType

N, T, CM, CX = 8, 4096, 64, 768
P = 128
T1 = T // 2          # 2048 tokens per side
TI = T1 // P         # 16 i-tiles (even side)
TJ = T1 // P         # 16 j-tiles (odd side)
NSLOT = 2 * T1       # 16 buckets x 256 slots
BIG = 1 << 20        # pad marker in the permutation table
SCALE = 16384.0      # 2^14: keeps the 2.7e-6 min gap fp16-normal

_CACHE = {}


def _build(debug=False):
    nc = bacc.Bacc("TRN2", target_bir_lowering=False, num_devices=N)
    metric_in = nc.declare_dram_parameter("metric", [T, CM], F32, isOutput=False)
    x_in = nc.declare_dram_parameter("x", [T, CX], F32, isOutput=False)
    out = nc.declare_dram_parameter("out", [T, CX], F32, isOutput=True)
    perm8 = nc.dram_tensor("perm8", [NSLOT, 2], I32)

    # token = (t*128 + p)*2 + e
    m_pv = metric_in[:].rearrange("(t p e) c -> e p t c", p=P, e=2)
    x_pv_odd = x_in[:].rearrange("(t p e) c -> e p t c", p=P, e=2)[1]
    out_r = out[:].rearrange("(t p e) c -> e t p c", p=P, e=2)
    perm_pv = perm8[:].rearrange("(u p) w -> p u w", p=P)

    with tile.TileContext(nc, num_cores=N) as tc:
        with tc.tile_pool(name="const", bufs=1) as cp:
            ident = cp.tile([P, P], F32)
            make_identity(nc, ident[:])
            iota_row = cp.tile([P, T1], F32)
            nc.gpsimd.iota(iota_row[:], pattern=[[1, T1]], base=0,
                           channel_multiplier=0,
                           allow_small_or_imprecise_dtypes=True)
            iota16 = cp.tile([P, 16], F32)
            nc.gpsimd.iota(iota16[:], pattern=[[1, 16]], base=0,
                           channel_multiplier=0,
                           allow_small_or_imprecise_dtypes=True)
            base4 = cp.tile([P, 4], F32)
            nc.gpsimd.iota(base4[:], pattern=[[512, 4]], base=0,
                           channel_multiplier=0,
                           allow_small_or_imprecise_dtypes=True)
            ones128 = cp.tile([P, P], F32)
            nc.vector.memset(ones128[:], 1.0)
            zeros8 = cp.tile([P, 8], F16)
            nc.vector.memset(zeros8[:], 0.0)
            ones2r = cp.tile([P, 2], F32R)
            nc.vector.tensor_copy(ones2r[:], ones128[:, 0:2])
            # UT[p', p] = 1.0 iff p' < p (strict upper as stored)
            ut_i = cp.tile([P, P], I32)
            nc.gpsimd.iota(ut_i[:], pattern=[[1, P]], base=0,
                           channel_multiplier=-1)
            utm = cp.tile([P, P], F32)
            nc.vector.tensor_scalar(utm[:], ut_i[:], 0, None, op0=OP.is_gt)
            # xrow[p, t] = 2p + 256t = DRAM row of even token (t*128+p)
            xrow_i32 = cp.tile([P, TI], I32)
            nc.gpsimd.iota(xrow_i32[:], pattern=[[256, TI]], base=0,
                           channel_multiplier=2)
            bigpat = cp.tile([P, 2 * NSLOT // P], I32)
            nc.vector.memset(bigpat[:], BIG)

            xodd_all = cp.tile([P, TI * CX], F32)
            idxf = cp.tile([P, TI], F32)
            idxu = cp.tile([P, TI], U32)
            slot_i32 = cp.tile([P, TI], I32)
            pr_all = cp.tile([P, TI * 2], I32)
            crun = cp.tile([P, 16], F32)       # running bucket counts (repl.)
            nc.vector.memset(crun[:], 0.0)

            # pre-fill the permutation table with the OOB marker
            nc.sync.dma_start(out=perm8[:].rearrange("(p u) w -> p (u w)", p=P),
                              in_=bigpat[:])

            with tc.tile_pool(name="work", bufs=1) as wp:
                aTpk = wp.tile([P, T1 // 2], F32)
                bTpk4 = [wp.tile([P, 512], F32, name=f"bTpk{q}") for q in range(4)]
                me = wp.tile([P, TI * CM], F32)
                mo = wp.tile([P, TI * CM], F32)

                nc.sync.dma_start(
                    out=xodd_all[:].rearrange("p (t c) -> p t c", c=CX),
                    in_=x_pv_odd)
                nc.sync.dma_start(out=me[:].rearrange("p (t c) -> p t c", c=CM),
                                  in_=m_pv[0])
                nc.sync.dma_start(out=mo[:].rearrange("p (t c) -> p t c", c=CM),
                                  in_=m_pv[1])

                with tc.tile_pool(name="pA", bufs=3) as pa, \
                     tc.tile_pool(name="psA", bufs=2, space="PSUM") as psa:

                    # ---- Phase A: normalize metric, transpose directly into
                    # the packed operands ----
                    def normalize(src, t):
                        mt = src[:, t * CM:(t + 1) * CM]
                        sq = pa.tile([P, CM], F32, tag="sq")
                        ssum = pa.tile([P, 1], F32, tag="ss")
                        nc.scalar.activation(sq[:], mt, AF.Square,
                                             accum_out=ssum[:])
                        nrm = pa.tile([P, 1], F32, tag="nr")
                        nc.scalar.sqrt(nrm[:], ssum[:])
                        rnm = pa.tile([P, 1], F32, tag="rn")
                        nc.vector.reciprocal(rnm[:], nrm[:])
                        nm = pa.tile([P, CM], F32, tag="nm")
                        nc.scalar.mul(nm[:], mt, rnm[:, 0:1])
                        return nm

                    for t in range(TI):          # odd side -> both bTpk halves
                        nm = normalize(mo, t)
                        pst = psa.tile([CM, P], F32, tag="tp", space="PSUM")
                        nc.tensor.transpose(pst[:], nm[:], ident[:])
                        blk = bTpk4[t // 4][:, (t % 4) * P:(t % 4 + 1) * P]
                        nc.scalar.copy(blk[0:CM, :], pst[:])
                        nc.sync.dma_start(out=blk[CM:P, :], in_=blk[0:CM, :])
                    for t in range(TI):          # even side -> aTpk half by parity
                        nm = normalize(me, t)
                        pst = psa.tile([CM, P], F32, tag="tp", space="PSUM")
                        nc.tensor.transpose(pst[:], nm[:], ident[:])
                        blk = aTpk[:, (t // 2) * P:(t // 2 + 1) * P]
                        if t % 2 == 0:
                            nc.scalar.copy(blk[0:CM, :], pst[:])
                        else:
                            stg = pa.tile([CM, P], F32, tag="stg")
                            nc.scalar.copy(stg[:], pst[:])
                            nc.sync.dma_start(out=blk[CM:P, :], in_=stg[:])

                with tc.tile_pool(name="pB", bufs=3) as pb, \
                     tc.tile_pool(name="psB", bufs=3, space="PSUM") as psb, \
                     tc.tile_pool(name="psR", bufs=2, space="PSUM") as psr:

                    # ---- Phase B: scores + chunk-local argmax + bucket slots
                    def rank_and_scatter(i):
                        """bucket, prefix-rank (PE), cross-tile count, slot,
                        scatter (row, idx) to perm8."""
                        bu = pb.tile([P, 1], U32, tag="bu")
                        nc.vector.tensor_scalar(bu[:], idxu[:, i:i + 1], 7,
                                                None,
                                                op0=OP.logical_shift_right)
                        bf = pb.tile([P, 1], F32, tag="bf")
                        nc.gpsimd.tensor_copy(bf[:], bu[:])
                        # one-hot over 16 buckets
                        oh = pb.tile([P, 16], F32, tag="oh")
                        nc.vector.tensor_scalar(oh[:], iota16[:], bf[:, 0:1],
                                                None, op0=OP.is_equal)
                        # within-tile rank via strict-UT prefix matmul:
                        # pc[p, b] = #(p' < p in bucket b); hist rides in the
                        # same PSUM bank
                        pch = psr.tile([P, 32], F32, tag="pc", space="PSUM")
                        nc.tensor.matmul(pch[:, 0:16], utm[:], oh[:],
                                         start=True, stop=True)
                        wcol = pb.tile([P, 1], F32, tag="wc")
                        junk = pb.tile([P, 16], F32, tag="jk")
                        nc.vector.scalar_tensor_tensor(
                            out=junk[:], in0=pch[:, 0:16], scalar=1.0,
                            in1=oh[:],
                            op0=OP.mult, op1=OP.mult, accum_out=wcol[:])
                        # cross-tile count so far (crun replicated per row)
                        ctv = pb.tile([P, 1], F32, tag="ctv")
                        junk2 = pb.tile([P, 16], F32, tag="jk2")
                        nc.vector.scalar_tensor_tensor(
                            out=junk2[:], in0=oh[:], scalar=1.0, in1=crun[:],
                            op0=OP.mult, op1=OP.mult, accum_out=ctv[:])
                        # crun += hist (replicated via ones matmul)
                        nc.tensor.matmul(pch[:, 16:32], ones128[:], oh[:],
                                         start=True, stop=True)
                        nc.vector.tensor_add(crun[:], crun[:], pch[:, 16:32])
                        # slot = 256*b + ctv + W
                        sf = pb.tile([P, 1], F32, tag="sf")
                        nc.vector.scalar_tensor_tensor(
                            out=sf[:], in0=bf[:], scalar=256.0, in1=ctv[:],
                            op0=OP.mult, op1=OP.add)
                        nc.vector.tensor_tensor(out=sf[:], in0=sf[:],
                                                in1=wcol[:], op=OP.add)
                        nc.vector.tensor_copy(slot_i32[:, i:i + 1], sf[:])
                        nc.vector.tensor_copy(pr_all[:, 2 * i:2 * i + 1],
                                              xrow_i32[:, i:i + 1])
                        nc.vector.tensor_copy(pr_all[:, 2 * i + 1:2 * i + 2],
                                              idxu[:, i:i + 1])
                        nc.gpsimd.indirect_dma_start(
                            out=perm8[:], in_=pr_all[:, 2 * i:2 * i + 2],
                            in_offset=None,
                            out_offset=IndirectOffsetOnAxis(
                                ap=slot_i32[:, i:i + 1], axis=0))

                    def argmax_tile(i, ps_chunks):
                        """ps_chunks: 4 PSUM aps [P,512].  Writes idxf/idxu."""
                        cm4 = pb.tile([P, 4], F32, tag="cm4")
                        resid = pb.tile([P, T1], F16, tag="resid")
                        mi8 = pb.tile([P, 32], U16, tag="mi8")
                        for c in range(4):
                            nc.vector.reduce_max(cm4[:, c:c + 1], ps_chunks[c],
                                                 axis=AX.X)
                            negb = pb.tile([P, 1], F32, tag=f"nb{c}")
                            nc.vector.tensor_scalar_mul(negb[:], cm4[:, c:c + 1],
                                                        -SCALE)
                            nc.scalar.activation(resid[:, 512 * c:512 * (c + 1)],
                                                 ps_chunks[c], AF.Identity,
                                                 scale=SCALE, bias=negb[:, 0:1])
                            nc.vector.max_index(
                                out=mi8[:, 8 * c:8 * c + 8],
                                in_max=zeros8[:],
                                in_values=resid[:, 512 * c:512 * (c + 1)])
                        gmax = pb.tile([P, 1], F32, tag="gm")
                        nc.vector.reduce_max(gmax[:], cm4[:], axis=AX.X)
                        oh4 = pb.tile([P, 4], F32, tag="oh4")
                        nc.vector.tensor_scalar(oh4[:], cm4[:], gmax[:, 0:1],
                                                None, op0=OP.is_equal)
                        uf4 = pb.tile([P, 4], F32, tag="uf4")
                        nc.vector.tensor_copy(
                            uf4[:], mi8[:].rearrange("p (c e) -> p c e", e=8)[:, :, 0])
                        ub4 = pb.tile([P, 4], F32, tag="ub4")
                        nc.vector.tensor_add(ub4[:], uf4[:], base4[:])
                        junk3 = pb.tile([P, 4], F32, tag="jk3")
                        nc.vector.scalar_tensor_tensor(
                            out=junk3[:], in0=ub4[:], scalar=1.0, in1=oh4[:],
                            op0=OP.mult, op1=OP.mult,
                            accum_out=idxf[:, i:i + 1])
                        nc.vector.tensor_copy(idxu[:, i:i + 1], idxf[:, i:i + 1])

                    for ii in range(TI // 2):
                        i0, i1 = 2 * ii, 2 * ii + 1
                        pchunks = [[], []]
                        for c in range(4):
                            pp = psb.tile([P, 1024], F32, tag="pp",
                                          space="PSUM")
                            nc.tensor.matmul(pp[:, 0:512],
                                             aTpk[0:CM, ii * P:(ii + 1) * P],
                                             bTpk4[c][0:CM, :],
                                             start=True, stop=True,
                                             tile_position=(0, 0))
                            nc.tensor.matmul(pp[:, 512:1024],
                                             aTpk[CM:P, ii * P:(ii + 1) * P],
                                             bTpk4[c][CM:P, :],
                                             start=True, stop=True,
                                             tile_position=(64, 0))
                            pchunks[0].append(pp[:, 0:512])
                            pchunks[1].append(pp[:, 512:1024])
                        for k, i in ((0, i0), (1, i1)):
                            argmax_tile(i, pchunks[k])
                            rank_and_scatter(i)

            # ---- Phase C+D: bucketed one-hot matmuls (f32r single pass),
            # dst write + transposed gather + even-row scatter, per j-tile ----
            with tc.tile_pool(name="pq", bufs=1) as pqp, \
                 tc.tile_pool(name="pC", bufs=4) as pcs, \
                 tc.tile_pool(name="pD", bufs=2) as pd, \
                 tc.tile_pool(name="psC", bufs=2, space="PSUM") as psc, \
                 tc.tile_pool(name="psD", bufs=2, space="PSUM") as psd:
                pq = pqp.tile([P, NSLOT // P * 2], I32)
                nc.sync.dma_start(
                    out=pq[:].rearrange("p (u w) -> p u w", w=2), in_=perm_pv)
                pq_v = pq[:].rearrange("p (u w) -> p u w", w=2)
                idxg_f = pqp.tile([P, NSLOT // P], F32)
                nc.vector.tensor_copy(idxg_f[:], pq_v[:, :, 1])
                qoff = pqp.tile([P, NSLOT // P], I32)
                nc.vector.tensor_copy(qoff[:], pq_v[:, :, 0])

                for jt in range(TJ):
                    psjn = psc.tile([P, 1024], F32, tag="sp", space="PSUM")
                    psj = psjn[:, 0:CX]
                    psn = psjn[:, CX:CX + 2]
                    eqrs = []
                    for k in range(2):
                        u = 2 * jt + k
                        xg = pcs.tile([P, CX], F32, tag="xg")
                        nc.gpsimd.indirect_dma_start(
                            out=xg[:], out_offset=None,
                            in_=x_in[:],
                            in_offset=IndirectOffsetOnAxis(
                                ap=qoff[:, u:u + 1], axis=0),
                            bounds_check=T - 1, oob_is_err=False)
                        xgr = pcs.tile([P, CX], F32R, tag="xgr")
                        nc.scalar.copy(xgr[:], xg[:])
                        eqr = pcs.tile([P, P], F32R, tag="eq")
                        nc.vector.tensor_scalar(
                            eqr[:], iota_row[:, jt * P:(jt + 1) * P],
                            idxg_f[:, u:u + 1], None, op0=OP.is_equal)
                        eqrs.append(eqr)
                        first, last = (k == 0), (k == 1)
                        nc.tensor.matmul(psn, eqr[:], ones2r[:],
                                         start=first, stop=last)
                        for lo_, hi_ in ((0, 512), (512, CX)):
                            nc.tensor.matmul(psj[:, lo_:hi_], eqr[:],
                                             xgr[:, lo_:hi_],
                                             start=first, stop=last)
                    xo = xodd_all[:, jt * CX:(jt + 1) * CX]
                    cnt1 = pd.tile([P, 1], F32, tag="c1")
                    nc.vector.tensor_scalar_add(cnt1[:], psn[:, 0:1], 1.0)
                    inv = pd.tile([P, 1], F32, tag="iv")
                    nc.vector.reciprocal(inv[:], cnt1[:])
                    dsum = pd.tile([P, CX], F32, tag="dsm")
                    nc.vector.tensor_add(dsum[:], xo, psj)
                    dst = pd.tile([P, CX], F32R, tag="dst")
                    nc.scalar.mul(dst[:], dsum[:], inv[:, 0:1])
                    nc.sync.dma_start(out=out_r[1, jt],
                                      in_=dst[:].bitcast(F32))
                    # ---- phase D folded in: gather dst rows for this
                    # j-tile's bucket and scatter to even out rows ----
                    for k in range(2):
                        u = 2 * jt + k
                        psgt = psd.tile([P, CX + P], F32, tag="gp",
                                        space="PSUM")
                        psg = psgt[:, 0:CX]
                        psT = psgt[:, CX:CX + P]
                        nc.tensor.transpose(psT, eqrs[k][:].bitcast(F32),
                                            ident[:])
                        eqrT = pd.tile([P, P], F32R, tag="eqT")
                        nc.scalar.copy(eqrT[:], psT)
                        for lo_, hi_ in ((0, 512), (512, CX)):
                            nc.tensor.matmul(psg[:, lo_:hi_], eqrT[:],
                                             dst[:, lo_:hi_],
                                             start=True, stop=True)
                        gout = pd.tile([P, CX], F32, tag="go")
                        nc.any.tensor_copy(gout[:], psg)
                        nc.gpsimd.indirect_dma_start(
                            out=out[:], in_=gout[:],
                            in_offset=None,
                            out_offset=IndirectOffsetOnAxis(
                                ap=qoff[:, u:u + 1], axis=0),
                            bounds_check=T - 1, oob_is_err=False)

    nc.compile()
    return nc


def kernel(metric: np.ndarray, x: np.ndarray) -> np.ndarray:
    if "nc" not in _CACHE:
        _CACHE["nc"] = _build()
    nc = _CACHE["nc"]
    metric = np.ascontiguousarray(np.asarray(metric, dtype=np.float32))
    x = np.ascontiguousarray(np.asarray(x, dtype=np.float32))
    in_maps = [{"metric": metric[c], "x": x[c]} for c in range(N)]
    res = run_bass_kernel_spmd(nc, in_maps, list(range(N)))
    return np.stack([res.results[c]["out"] for c in range(N)], axis=0)


# revision 17
# speedup vs baseline: 1.1243x; 1.1243x over previous
"""Bass/Trainium2 kernel for nn_BipartiteSoftMatching (8 cores, batch-parallel).

Since r = t//2 the argsort in the reference is irrelevant: src_idx is a full
permutation and unm_idx is empty.  Per batch element the computation reduces to
  m = metric / ||metric||;  scores = m_even @ m_odd^T
  node_idx[i] = argmax_j scores[i, j]
  dst_out[j]  = (x_odd[j] + sum_{i: node_idx[i]=j} x_even[i]) / (1 + count[j])
  out[2j+1]   = dst_out[j];   out[2i] = dst_out[node_idx[i]]

Hardware mapping (v1 rewrite):
  - scores: fp32 matmuls (exact; min top-2 gap on this data is 2.7e-6),
    two i-tiles packed into the 128-row array via tile_position halves.
  - argmax: per 512-chunk, DVE reduce_max straight from PSUM, then an ACT
    copy emits fp16((s - cmax) * 2^14) (exact 0 only at the chunk argmax;
    the 2^14 scale keeps the min gap in fp16-normal range) and DVE
    find_index8 searches 0.0 on 2-byte data.  A tiny one-hot combine picks
    the winning chunk.  This replaces two full fp32 scans per tile.
  - scatter-add, bucketed: tokens are permuted into 16 buckets by dest
    j-tile (256 slots, OOB-padded).  slot = 256*bucket + cross-tile-count
    + within-tile-rank, where the within-tile rank comes from a strict
    upper-triangular prefix matmul (PE) instead of a transpose trick.
    (row, idx) pairs scatter to a DRAM perm table; x_even rows gather back
    bucket-contiguous with OOB rows skipped via bounds_check.
  - phase C: one f32r one-hot matmul pass per half-bucket (tolerance is
    2e-2; f32r's ~2^-12 rounding is plenty), counts ride along as a
    2-column ones matmul.
  - phase D is folded into phase C per j-tile: the transposed one-hot
    gathers dst rows with a second f32r matmul and an indirect scatter
    writes them straight to the even output rows (no DRAM staging buffer).
"""

import numpy as np

import concourse.bacc as bacc
import concourse.bass as bass
import concourse.mybir as mybir
import concourse.tile as tile
from concourse.bass import IndirectOffsetOnAxis
from concourse.bass_utils import run_bass_kernel_spmd
from concourse.masks import make_identity

F32 = mybir.dt.float32
F32R = mybir.dt.float32r
F16 = mybir.dt.float16
U16 = mybir.dt.uint16
U32 = mybir.dt.uint32
I32 = mybir.dt.int32
OP = mybir.AluOpType
AF = mybir.ActivationFunctionType
AX = mybir.AxisListType

N, T, CM, CX = 8, 4096, 64, 768
P = 128
T1 = T // 2          # 2048 tokens per side
TI = T1 // P         # 16 i-tiles (even side)
TJ = T1 // P         # 16 j-tiles (odd side)
NSLOT = 2 * T1       # 16 buckets x 256 slots
BIG = 1 << 20        # pad marker in the permutation table
SCALE = 16384.0      # 2^14: keeps the 2.7e-6 min gap fp16-normal

_CACHE = {}


def _build(debug=False):
    nc = bacc.Bacc("TRN2", target_bir_lowering=False, num_devices=N)
    metric_in = nc.declare_dram_parameter("metric", [T, CM], F32, isOutput=False)
    x_in = nc.declare_dram_parameter("x", [T, CX], F32, isOutput=False)
    out = nc.declare_dram_parameter("out", [T, CX], F32, isOutput=True)
    if debug:
        perm8 = nc.declare_dram_parameter("perm8", [NSLOT, 2], I32,
                                          isOutput=True)
        idx_dbg = nc.declare_dram_parameter("idx_dbg", [P, TI], F32,
                                            isOutput=True)
    else:
        perm8 = nc.dram_tensor("perm8", [NSLOT, 2], I32)

    # token = (t*128 + p)*2 + e
    m_pv = metric_in[:].rearrange("(t p e) c -> e p t c", p=P, e=2)
    x_pv_odd = x_in[:].rearrange("(t p e) c -> e p t c", p=P, e=2)[1]
    out_r = out[:].rearrange("(t p e) c -> e t p c", p=P, e=2)
    perm_pv = perm8[:].rearrange("(u p) w -> p u w", p=P)

    with tile.TileContext(nc, num_cores=N) as tc:
        with tc.tile_pool(name="const", bufs=1) as cp:
            ident = cp.tile([P, P], F32)
            make_identity(nc, ident[:])
            iota_row = cp.tile([P, T1], F32)
            nc.gpsimd.iota(iota_row[:], pattern=[[1, T1]], base=0,
                           channel_multiplier=0,
                           allow_small_or_imprecise_dtypes=True)
            iota16 = cp.tile([P, 16], F32)
            nc.gpsimd.iota(iota16[:], pattern=[[1, 16]], base=0,
                           channel_multiplier=0,
                           allow_small_or_imprecise_dtypes=True)
            base4 = cp.tile([P, 4], F32)
            nc.gpsimd.iota(base4[:], pattern=[[512, 4]], base=0,
                           channel_multiplier=0,
                           allow_small_or_imprecise_dtypes=True)
            ones128 = cp.tile([P, P], F32)
            nc.vector.memset(ones128[:], 1.0)
            zeros8 = cp.tile([P, 8], F16)
            nc.vector.memset(zeros8[:], 0.0)
            ones2r = cp.tile([P, 2], F32R)
            nc.vector.tensor_copy(ones2r[:], ones128[:, 0:2])
            # UT[p', p] = 1.0 iff p' < p (strict upper as stored)
            ut_i = cp.tile([P, P], I32)
            nc.gpsimd.iota(ut_i[:], pattern=[[1, P]], base=0,
                           channel_multiplier=-1)
            utm = cp.tile([P, P], F32)
            nc.vector.tensor_scalar(utm[:], ut_i[:], 0, None, op0=OP.is_gt)
            # xrow[p, t] = 2p + 256t = DRAM row of even token (t*128+p)
            xrow_i32 = cp.tile([P, TI], I32)
            nc.gpsimd.iota(xrow_i32[:], pattern=[[256, TI]], base=0,
                           channel_multiplier=2)
            bigpat = cp.tile([P, 2 * NSLOT // P], I32)
            nc.vector.memset(bigpat[:], BIG)

            xodd_all = cp.tile([P, TI * CX], F32)
            idxf = cp.tile([P, TI], F32)
            idxu = cp.tile([P, TI], U32)
            slot_i32 = cp.tile([P, TI], I32)
            pr_all = cp.tile([P, TI * 2], I32)
            crun = cp.tile([P, 16], F32)       # running bucket counts (repl.)
            nc.vector.memset(crun[:], 0.0)

            # pre-fill the permutation table with the OOB marker
            nc.sync.dma_start(out=perm8[:].rearrange("(p u) w -> p (u w)", p=P),
                              in_=bigpat[:])

            with tc.tile_pool(name="work", bufs=1) as wp:
                aTpk = wp.tile([P, T1 // 2], F32)
                bTpk4 = [wp.tile([P, 512], F32, name=f"bTpk{q}") for q in range(4)]
                me = wp.tile([P, TI * CM], F32)
                mo = wp.tile([P, TI * CM], F32)

                nc.sync.dma_start(
                    out=xodd_all[:].rearrange("p (t c) -> p t c", c=CX),
                    in_=x_pv_odd)
                nc.sync.dma_start(out=me[:].rearrange("p (t c) -> p t c", c=CM),
                                  in_=m_pv[0])
                nc.sync.dma_start(out=mo[:].rearrange("p (t c) -> p t c", c=CM),
                                  in_=m_pv[1])

                with tc.tile_pool(name="pA", bufs=3) as pa, \
                     tc.tile_pool(name="psA", bufs=2, space="PSUM") as psa:

                    # ---- Phase A: normalize metric, transpose directly into
                    # the packed operands ----
                    def normalize(src, t):
                        mt = src[:, t * CM:(t + 1) * CM]
                        sq = pa.tile([P, CM], F32, tag="sq")
                        ssum = pa.tile([P, 1], F32, tag="ss")
                        nc.scalar.activation(sq[:], mt, AF.Square,
                                             accum_out=ssum[:])
                        nrm = pa.tile([P, 1], F32, tag="nr")
                        nc.scalar.sqrt(nrm[:], ssum[:])
                        rnm = pa.tile([P, 1], F32, tag="rn")
                        nc.vector.reciprocal(rnm[:], nrm[:])
                        nm = pa.tile([P, CM], F32, tag="nm")
                        nc.scalar.mul(nm[:], mt, rnm[:, 0:1])
                        return nm

                    for t in range(TI):          # odd side -> both bTpk halves
                        nm = normalize(mo, t)
                        pst = psa.tile([CM, P], F32, tag="tp", space="PSUM")
                        nc.tensor.transpose(pst[:], nm[:], ident[:])
                        blk = bTpk4[t // 4][:, (t % 4) * P:(t % 4 + 1) * P]
                        nc.scalar.copy(blk[0:CM, :], pst[:])
                        nc.sync.dma_start(out=blk[CM:P, :], in_=blk[0:CM, :])
                    for t in range(TI):          # even side -> aTpk half by parity
                        nm = normalize(me, t)
                        pst = psa.tile([CM, P], F32, tag="tp", space="PSUM")
                        nc.tensor.transpose(pst[:], nm[:], ident[:])
                        blk = aTpk[:, (t // 2) * P:(t // 2 + 1) * P]
                        if t % 2 == 0:
                            nc.scalar.copy(blk[0:CM, :], pst[:])
                        else:
                            stg = pa.tile([CM, P], F32, tag="stg")
                            nc.scalar.copy(stg[:], pst[:])
                            nc.sync.dma_start(out=blk[CM:P, :], in_=stg[:])

                with tc.tile_pool(name="pB", bufs=3) as pb, \
                     tc.tile_pool(name="psB", bufs=3, space="PSUM") as psb, \
                     tc.tile_pool(name="psR", bufs=2, space="PSUM") as psr:

                    # ---- Phase B: scores + chunk-local argmax + bucket slots
                    def rank_and_scatter(i):
                        """bucket, prefix-rank (PE), cross-tile count, slot,
                        scatter (row, idx) to perm8."""
                        bu = pb.tile([P, 1], U32, tag="bu")
                        nc.vector.tensor_scalar(bu[:], idxu[:, i:i + 1], 7,
                                                None,
                                                op0=OP.logical_shift_right)
                        bf = pb.tile([P, 1], F32, tag="bf")
                        nc.gpsimd.tensor_copy(bf[:], bu[:])
                        # one-hot over 16 buckets
                        oh = pb.tile([P, 16], F32, tag="oh")
                        nc.vector.tensor_scalar(oh[:], iota16[:], bf[:, 0:1],
                                                None, op0=OP.is_equal)
                        # within-tile rank via strict-UT prefix matmul:
                        # pc[p, b] = #(p' < p in bucket b); hist rides in the
                        # same PSUM bank
                        pch = psr.tile([P, 32], F32, tag="pc", space="PSUM")
                        nc.tensor.matmul(pch[:, 0:16], utm[:], oh[:],
                                         start=True, stop=True)
                        wcol = pb.tile([P, 1], F32, tag="wc")
                        junk = pb.tile([P, 16], F32, tag="jk")
                        nc.vector.scalar_tensor_tensor(
                            out=junk[:], in0=pch[:, 0:16], scalar=1.0,
                            in1=oh[:],
                            op0=OP.mult, op1=OP.mult, accum_out=wcol[:])
                        # cross-tile count so far (crun replicated per row)
                        ctv = pb.tile([P, 1], F32, tag="ctv")
                        junk2 = pb.tile([P, 16], F32, tag="jk2")
                        nc.vector.scalar_tensor_tensor(
                            out=junk2[:], in0=oh[:], scalar=1.0, in1=crun[:],
                            op0=OP.mult, op1=OP.mult, accum_out=ctv[:])
                        # crun += hist (replicated via ones matmul)
                        nc.tensor.matmul(pch[:, 16:32], ones128[:], oh[:],
                                         start=True, stop=True)
                        nc.vector.tensor_add(crun[:], crun[:], pch[:, 16:32])
                        # slot = 256*b + ctv + W
                        sf = pb.tile([P, 1], F32, tag="sf")
                        nc.vector.scalar_tensor_tensor(
                            out=sf[:], in0=bf[:], scalar=256.0, in1=ctv[:],
                            op0=OP.mult, op1=OP.add)
                        nc.vector.tensor_tensor(out=sf[:], in0=sf[:],
                                                in1=wcol[:], op=OP.add)
                        nc.vector.tensor_copy(slot_i32[:, i:i + 1], sf[:])
                        nc.vector.tensor_copy(pr_all[:, 2 * i:2 * i + 1],
                                              xrow_i32[:, i:i + 1])
                        nc.vector.tensor_copy(pr_all[:, 2 * i + 1:2 * i + 2],
                                              idxu[:, i:i + 1])
                        nc.gpsimd.indirect_dma_start(
                            out=perm8[:], in_=pr_all[:, 2 * i:2 * i + 2],
                            in_offset=None,
                            out_offset=IndirectOffsetOnAxis(
                                ap=slot_i32[:, i:i + 1], axis=0))

                    def argmax_tile(i, ps_chunks):
                        """ps_chunks: 4 PSUM aps [P,512].  Writes idxf/idxu."""
                        cm4 = pb.tile([P, 4], F32, tag="cm4")
                        resid = pb.tile([P, T1], F16, tag="resid")
                        mi8 = pb.tile([P, 32], U16, tag="mi8")
                        for c in range(4):
                            nc.vector.reduce_max(cm4[:, c:c + 1], ps_chunks[c],
                                                 axis=AX.X)
                            negb = pb.tile([P, 1], F32, tag=f"nb{c}")
                            nc.vector.tensor_scalar_mul(negb[:], cm4[:, c:c + 1],
                                                        -SCALE)
                            nc.scalar.activation(resid[:, 512 * c:512 * (c + 1)],
                                                 ps_chunks[c], AF.Identity,
                                                 scale=SCALE, bias=negb[:, 0:1])
                            nc.vector.max_index(
                                out=mi8[:, 8 * c:8 * c + 8],
                                in_max=zeros8[:],
                                in_values=resid[:, 512 * c:512 * (c + 1)])
                        gmax = pb.tile([P, 1], F32, tag="gm")
                        nc.vector.reduce_max(gmax[:], cm4[:], axis=AX.X)
                        oh4 = pb.tile([P, 4], F32, tag="oh4")
                        nc.vector.tensor_scalar(oh4[:], cm4[:], gmax[:, 0:1],
                                                None, op0=OP.is_equal)
                        uf4 = pb.tile([P, 4], F32, tag="uf4")
                        nc.vector.tensor_copy(
                            uf4[:], mi8[:].rearrange("p (c e) -> p c e", e=8)[:, :, 0])
                        ub4 = pb.tile([P, 4], F32, tag="ub4")
                        nc.vector.tensor_add(ub4[:], uf4[:], base4[:])
                        junk3 = pb.tile([P, 4], F32, tag="jk3")
                        nc.vector.scalar_tensor_tensor(
                            out=junk3[:], in0=ub4[:], scalar=1.0, in1=oh4[:],
                            op0=OP.mult, op1=OP.mult,
                            accum_out=idxf[:, i:i + 1])
                        nc.vector.tensor_copy(idxu[:, i:i + 1], idxf[:, i:i + 1])

                    for ii in range(TI // 2):
                        i0, i1 = 2 * ii, 2 * ii + 1
                        pchunks = [[], []]
                        for c in range(4):
                            pp = psb.tile([P, 1024], F32, tag="pp",
                                          space="PSUM")
                            nc.tensor.matmul(pp[:, 0:512],
                                             aTpk[0:CM, ii * P:(ii + 1) * P],
                                             bTpk4[c][0:CM, :],
                                             start=True, stop=True,
                                             tile_position=(0, 0))
                            nc.tensor.matmul(pp[:, 512:1024],
                                             aTpk[CM:P, ii * P:(ii + 1) * P],
                                             bTpk4[c][CM:P, :],
                                             start=True, stop=True,
                                             tile_position=(64, 0))
                            pchunks[0].append(pp[:, 0:512])
                            pchunks[1].append(pp[:, 512:1024])
                        for k, i in ((0, i0), (1, i1)):
                            argmax_tile(i, pchunks[k])
                            rank_and_scatter(i)

            if debug:
                nc.sync.dma_start(out=idx_dbg[:], in_=idxf[:])
            # ---- Phase C+D: bucketed one-hot matmuls (f32r single pass),
            # dst write + transposed gather + even-row scatter, per j-tile ----
            with tc.tile_pool(name="pq", bufs=1) as pqp, \
                 tc.tile_pool(name="pC", bufs=4) as pcs, \
                 tc.tile_pool(name="pD", bufs=2) as pd, \
                 tc.tile_pool(name="psC", bufs=2, space="PSUM") as psc, \
                 tc.tile_pool(name="psD", bufs=1, space="PSUM") as psd:
                pq = pqp.tile([P, NSLOT // P * 2], I32)
                nc.sync.dma_start(
                    out=pq[:].rearrange("p (u w) -> p u w", w=2), in_=perm_pv)
                pq_v = pq[:].rearrange("p (u w) -> p u w", w=2)
                idxg_f = pqp.tile([P, NSLOT // P], F32)
                nc.vector.tensor_copy(idxg_f[:], pq_v[:, :, 1])
                qoff = pqp.tile([P, NSLOT // P], I32)
                nc.vector.tensor_copy(qoff[:], pq_v[:, :, 0])

                for jt in range(TJ):
                    psjn = psc.tile([P, 1024], F32, tag="sp", space="PSUM")
                    psj = psjn[:, 0:CX]
                    eqrs = []
                    for k in range(2):
                        u = 2 * jt + k
                        xg = pcs.tile([P, CX], F32, tag="xg")
                        nc.gpsimd.indirect_dma_start(
                            out=xg[:], out_offset=None,
                            in_=x_in[:],
                            in_offset=IndirectOffsetOnAxis(
                                ap=qoff[:, u:u + 1], axis=0),
                            bounds_check=T - 1, oob_is_err=False)
                        xgr = pcs.tile([P, CX], F32R, tag="xgr")
                        nc.scalar.copy(xgr[:], xg[:])
                        eqr = pcs.tile([P, P], F32R, tag="eq")
                        nc.vector.tensor_scalar(
                            eqr[:], iota_row[:, jt * P:(jt + 1) * P],
                            idxg_f[:, u:u + 1], None, op0=OP.is_equal)
                        eqrs.append(eqr)
                        first, last = (k == 0), (k == 1)
                        for lo_, hi_ in ((0, 512), (512, CX)):
                            nc.tensor.matmul(psj[:, lo_:hi_], eqr[:],
                                             xgr[:, lo_:hi_],
                                             start=first, stop=last)
                    # transposed one-hots (also used for counts)
                    psgts, eqrTs = [], []
                    for k in range(2):
                        psgt = psd.tile([P, CX + P], F32, tag=f"gp{k}",
                                        space="PSUM")
                        psT = psgt[:, CX:CX + P]
                        nc.tensor.transpose(psT, eqrs[k][:].bitcast(F32),
                                            ident[:])
                        eqrT = pd.tile([P, P], F32R, tag=f"eqT{k}")
                        nc.scalar.copy(eqrT[:], psT)
                        psgts.append(psgt)
                        eqrTs.append(eqrT)
                    c0 = pd.tile([P, 2], F32, tag="c0")
                    nc.vector.reduce_sum(c0[:, 0:1], eqrTs[0][:].bitcast(F32),
                                         axis=AX.X)
                    nc.vector.reduce_sum(c0[:, 1:2], eqrTs[1][:].bitcast(F32),
                                         axis=AX.X)
                    cnt1 = pd.tile([P, 1], F32, tag="c1")
                    nc.vector.scalar_tensor_tensor(
                        out=cnt1[:], in0=c0[:, 0:1], scalar=1.0,
                        in1=c0[:, 1:2], op0=OP.add, op1=OP.add)
                    inv = pd.tile([P, 1], F32, tag="iv")
                    nc.vector.reciprocal(inv[:], cnt1[:])
                    xo = xodd_all[:, jt * CX:(jt + 1) * CX]
                    dsum = pd.tile([P, CX], F32, tag="dsm")
                    nc.vector.tensor_add(dsum[:], xo, psj)
                    dst = pd.tile([P, CX], F32R, tag="dst")
                    nc.scalar.mul(dst[:], dsum[:], inv[:, 0:1])
                    nc.sync.dma_start(out=out_r[1, jt],
                                      in_=dst[:].bitcast(F32))
                    # ---- phase D folded in: gather dst rows for this
                    # j-tile's bucket and scatter to even out rows ----
                    for k in range(2):
                        u = 2 * jt + k
                        psg = psgts[k][:, 0:CX]
                        for lo_, hi_ in ((0, 512), (512, CX)):
                            nc.tensor.matmul(psg[:, lo_:hi_], eqrTs[k][:],
                                             dst[:, lo_:hi_],
                                             start=True, stop=True)
                        gout = pd.tile([P, CX], F32, tag="go")
                        nc.any.tensor_copy(gout[:], psg)
                        nc.gpsimd.indirect_dma_start(
                            out=out[:], in_=gout[:],
                            in_offset=None,
                            out_offset=IndirectOffsetOnAxis(
                                ap=qoff[:, u:u + 1], axis=0),
                            bounds_check=T - 1, oob_is_err=False)

    nc.compile()
    return nc


def kernel(metric: np.ndarray, x: np.ndarray) -> np.ndarray:
    if "nc" not in _CACHE:
        _CACHE["nc"] = _build()
    nc = _CACHE["nc"]
    metric = np.ascontiguousarray(np.asarray(metric, dtype=np.float32))
    x = np.ascontiguousarray(np.asarray(x, dtype=np.float32))
    in_maps = [{"metric": metric[c], "x": x[c]} for c in range(N)]
    res = run_bass_kernel_spmd(nc, in_maps, list(range(N)))
    return np.stack([res.results[c]["out"] for c in range(N)], axis=0)
